# revision 24
# baseline (speedup 1.0000x reference)
"""ArcFace loss with adaptive margins and subcenters, distributed over 8 TRN2 cores.

Problem: features [512, 512] f32, weight [300000, 512] f32 (100000 classes x 3
subcenters), margins [100000] f32, labels [512] int. Output [512, 100000] f32:
S * max_k cos(f, w_{c,k}) everywhere, with the ArcFace margin phi at each
sample's label column.

Strategy (classifier/model parallel, per the class-sharding hint):
  - Host: L2-normalize features and weights, fold the scale S into the weight,
    pack each core's 12500-class shard into a DMA-friendly layout, and compute
    (exactly, in f32) the per-sample label-column value phi.
  - Device (x8, no collectives): stream the packed weight shard from HBM.
    16/25 of each core's class-chunks run as fp8e4m3 matmuls in DoubleRow perf
    mode (256-deep contraction per instruction, measured 1.94-1.97x the fp16
    MAC rate on HW); the other 9/25 run in fp16. Plain RTN fp8 would give
    rel-err 2.9e-2 if applied everywhere; adaptive rounding (coordinate
    descent on the rounding choices against the known counterpart matrix,
    AdaRound-style) plus the fp16 majority lands ~1.8e-2 < 2e-2.
    fp8 and fp16 chunks are interleaved so the constant-rate PSUM-drain
    pipeline (ACT copy + 2 DVE maxes, ~1.35us/block) stays below the mean PE
    cadence. Elementwise max over the 3 subcenters, fp16 cosine shard to HBM
    (f32 stores would saturate the ~358 GB/s HBM port alongside the weight
    stream), host upcasts.
  - Host: concatenate the 8 shards and overwrite the label entries with S*phi.

Per-core PE: 600k column-cycles at fp16 would be 250us; the hybrid needs
9*24k + 16*12k = 408k cycles ~ 170us. HBM traffic ~48MB ~ 135us.
"""

import numpy as np

B = 512            # batch
D = 512            # in_features
C = 100000         # n_classes
K = 3              # subcenters
S = 30.0           # ArcFace scale
NCORES = 8
CPC = C // NCORES  # classes per core = 12500
NCHUNK = 500       # output columns per PSUM tile
CHUNKS = CPC // NCHUNK   # 25
NC8 = 16           # chunks computed in fp8 DoubleRow (cols 0..NC8*500)
NC16 = CHUNKS - NC8
NB = B // 128      # 4 row blocks of the batch
DBLK = D // 128    # 4 contraction blocks (fp16); 2 pairs of 256 (fp8 DR)
RU = NC8 * NCHUNK * K  # ada-rounded weight rows per core

_CACHE = {}
LAST_RESULT = None  # BassKernelResults of the most recent run (for profiling)


def _install_profile_hook():
    """Make `antenv.axon_hooks` importable (concourse imports it when tracing
    is requested via BASS_TRACE) and register the NTFF hook if available."""
    import sys
    import types
    try:
        import antenv
    except ImportError:
        return
    if getattr(antenv, "axon_hooks", None) is not None:
        return
    mod = types.ModuleType("antenv.axon_hooks")
    _hook = [None]
    mod.set_axon_ntff_profile_hook = lambda h: _hook.__setitem__(0, h)
    mod.get_axon_ntff_profile_hook = lambda: _hook[0]
    sys.modules["antenv.axon_hooks"] = mod
    antenv.axon_hooks = mod
    try:
        from trn_agent_boot.trn_boot import _ntff_profile_via_ctypes
        hook = _ntff_profile_via_ctypes("/opt/axon/libaxon_pjrt.so")
        if hook is not None:
            mod.set_axon_ntff_profile_hook(hook)
    except Exception:
        pass


def _build_nc():
    if "nc" in _CACHE:
        return _CACHE["nc"]
    import concourse.bacc as bacc
    import concourse.tile as tile
    from concourse import mybir

    BF = mybir.dt.float16
    E4 = mybir.dt.float8e4
    F32 = mybir.dt.float32
    DRM = mybir.MatmulPerfMode.DoubleRow

    nc = bacc.Bacc("TRN2", target_bir_lowering=False, debug=False, num_devices=NCORES)
    # fp8 weight shard, DoubleRow moving layout:
    #   w8[q][p][((k*2+P)*2+i)*NCHUNK + j] = S * wn[3*(c0+q*500+j)+k, P*256+i*128+p]
    w8 = nc.dram_tensor("w8", [NC8, 128, K * 2 * 2 * NCHUNK], E4, kind="ExternalInput")
    # fp16 packed weight shard (chunks NC8..24):
    #   wt[q][p][(k*DBLK+d)*NCHUNK + j] = S * wn[3*(c0+(NC8+q)*500+j)+k, d*128+p]
    wt = nc.dram_tensor("wt", [NC16, 128, K * DBLK * NCHUNK], BF, kind="ExternalInput")
    # Normalized features, fp16 transposed: fnT[d][p][b] = fn[b, d*128+p]
    fnT = nc.dram_tensor("fnT", [DBLK, 128, B], BF, kind="ExternalInput")
    # fp8 features, DoubleRow stationary layout: f8[p][P][i][b] = fn[b, P*256+i*128+p]
    f8 = nc.dram_tensor("f8", [128, 2, 2, B], E4, kind="ExternalInput")
    # fp16 output halves the store traffic (f32 stores + the weight stream
    # would exceed ~358 GB/s during fp8 chunks); host upcasts to f32.
    out = nc.dram_tensor("out", [B, CPC], BF, kind="ExternalOutput")

    with tile.TileContext(nc, trace_sim=False) as tc:
        with tc.tile_pool(name="fp", bufs=1) as fpool, \
             tc.tile_pool(name="wp", bufs=4) as wpool, \
             tc.tile_pool(name="w8p", bufs=4) as w8pool, \
             tc.tile_pool(name="op", bufs=3) as opool, \
             tc.tile_pool(name="tp", bufs=4) as tpool, \
             tc.tile_pool(name="pp0", bufs=2, space="PSUM") as ppool0, \
             tc.tile_pool(name="pp1", bufs=2, space="PSUM") as ppool1, \
             tc.tile_pool(name="pp2", bufs=4, space="PSUM") as ppool2:
            f_sb = fpool.tile([128, DBLK * B], BF)
            f8_sb = fpool.tile([128, 2, 2, B], E4)  # [p, P, i, b]

            def max_store(ps, b, q, ob_big, last=False):
                # PSUM drain: ACT stages k=0 (the DVE can't read two PSUM
                # banks in one op), DVE does the two maxes (~1.35us/block);
                # the fp8/fp16 interleave keeps the mean PE cadence above it.
                t0 = tpool.tile([128, NCHUNK], F32, tag="t0", name="t0")
                nc.scalar.copy(t0[:], ps[0][:])
                t01 = tpool.tile([128, NCHUNK], F32, tag="t01", name="t01")
                nc.vector.tensor_max(t01[:], t0[:], ps[1][:])
                nc.vector.tensor_max(ob_big[:, b], t01[:], ps[2][:])
                if last:
                    # Final chunk: store per b-block so the kernel tail
                    # overlaps the drain instead of one big end store.
                    nc.sync.dma_start(
                        out[b * 128:(b + 1) * 128, q * NCHUNK:(q + 1) * NCHUNK],
                        ob_big[:, b],
                    )
                elif b == NB - 1:
                    # One batched store per chunk (a dma_start occupies the
                    # issuing engine ~600ns, so fewer+larger stores), on
                    # sync's HWDGE ring; weights ride gpsimd's ring.
                    nc.sync.dma_start(
                        out[:, q * NCHUNK:(q + 1) * NCHUNK]
                        .rearrange("(bb p) c -> p bb c", bb=NB),
                        ob_big[:],
                    )

            def psum_tiles():
                pools = (ppool0, ppool1, ppool2)
                return [
                    pools[k].tile([128, NCHUNK], F32, tag=f"ps{k}", name=f"ps{k}")
                    for k in range(K)
                ]

            def emit_f8(q, last=False):
                w_sb = w8pool.tile([128, K * 2, 2, NCHUNK], E4)
                if q == 0:
                    # First chunk: the first DR matmul needs only the P=0
                    # feature slice + the (k0,P0) weight slice; issue those
                    # first on separate rings, then the rest in consumption
                    # order.
                    w83 = w8[q].rearrange("p (m two j) -> p m two j", m=K * 2, two=2)
                    nc.scalar.dma_start(f8_sb[:, 0], f8[:, 0])
                    nc.sync.dma_start(w_sb[:, 0], w83[:, 0])
                    nc.scalar.dma_start(f8_sb[:, 1], f8[:, 1])
                    nc.gpsimd.dma_start(w_sb[:, 1], w83[:, 1])
                    for m in range(2, K * 2):
                        eng = nc.sync if m % 2 == 0 else nc.gpsimd
                        eng.dma_start(w_sb[:, m], w83[:, m])
                    for d in range(DBLK):
                        nc.scalar.dma_start(f_sb[:, d * B:(d + 1) * B], fnT[d])
                else:
                    nc.gpsimd.dma_start(w_sb[:], w8[q])
                ob_big = opool.tile([128, NB, NCHUNK], BF, tag="ob", name="ob")
                for b in range(NB):
                    ps = psum_tiles()
                    for k in range(K):
                        for P in range(2):
                            lh = f8_sb[:, P, :, b * 128:(b + 1) * 128]
                            rh = w_sb[:, k * 2 + P]
                            nc.tensor.matmul(
                                ps[k][:], lh, rh,
                                start=(P == 0), stop=(P == 1),
                                perf_mode=DRM,
                                skip_group_check=True,
                            )
                    max_store(ps, b, q, ob_big, last)

            def emit_f16(q, last=False):
                w_sb = wpool.tile([128, K * DBLK * NCHUNK], BF)
                nc.gpsimd.dma_start(w_sb[:], wt[q])
                ob_big = opool.tile([128, NB, NCHUNK], BF, tag="ob", name="ob")
                for b in range(NB):
                    ps = psum_tiles()
                    # d-outer / k-inner: the stationary operand (features)
                    # is reused across the 3 subcenter matmuls.
                    for d in range(DBLK):
                        lh = f_sb[:, d * B + b * 128: d * B + (b + 1) * 128]
                        for k in range(K):
                            rh = w_sb[:, (k * DBLK + d) * NCHUNK:(k * DBLK + d + 1) * NCHUNK]
                            nc.tensor.matmul(
                                ps[k][:], lh, rh,
                                start=(d == 0), stop=(d == DBLK - 1),
                                skip_group_check=True,
                            )
                    max_store(ps, b, NC8 + q, ob_big, last)

            # Interleave fp8 (5us PE/chunk) and fp16 (10us) chunks (Bresenham
            # merge) so the constant-rate PSUM drain never falls behind.
            order = []
            acc = 0
            i8 = i16 = 0
            for _ in range(CHUNKS):
                acc += NC16
                if acc >= CHUNKS and i16 < NC16:
                    acc -= CHUNKS
                    order.append(("f16", i16))
                    i16 += 1
                elif i8 < NC8:
                    order.append(("f8", i8))
                    i8 += 1
                else:
                    order.append(("f16", i16))
                    i16 += 1
            for i, (kind, q) in enumerate(order):
                (emit_f8 if kind == "f8" else emit_f16)(q, last=(i == len(order) - 1))
    nc.compile()
    _CACHE["nc"] = nc
    return nc


def _to_f16(x):
    # fp16 storage/compute: same TensorE rate and DMA bytes as bf16, but a
    # 10-bit mantissa -> ~4x less quantization error. All values here are
    # bounded by S=30, far inside fp16 range.
    return np.asarray(x, np.float32).astype(np.float16)


def _e4():
    import ml_dtypes
    return ml_dtypes.float8_e4m3


def _q8(x):
    return x.astype(_e4()).astype(np.float32)


def _other_candidate(x, q):
    """e4m3 grid neighbor of q on the other side of x (toward x)."""
    u = x.astype(_e4()).view(np.uint8).astype(np.int16)
    xf = np.asarray(x, np.float32)
    delta = np.where((xf > q) == (q >= 0), 1, -1).astype(np.int16)
    alt = (u + delta).clip(0, 255).astype(np.uint8).view(_e4()).astype(np.float32)
    return np.where(xf == q, q, alt)


def _cd_exact(F, M, sweeps=5):
    """AdaRound via exact coordinate descent: choose the e4m3 rounding of
    each F entry to minimize tr(E M E^T), E = F - Q, M = gram matrix of the
    counterpart operand. Small row count -> exact sequential CD is cheap."""
    q = _q8(F)
    alt = _other_candidate(F, q)
    E = F - q
    G = E @ M
    Md = np.diag(M).copy()
    for _ in range(sweeps):
        flips = 0
        for dd in range(F.shape[1]):
            t = q[:, dd] - alt[:, dd]
            delta = 2.0 * t * G[:, dd] + t * t * Md[dd]
            mask = delta < 0
            if not mask.any():
                continue
            flips += int(mask.sum())
            tm = np.where(mask, t, 0.0)
            E[:, dd] += tm
            G += np.outer(tm, M[dd])
            qa = q[:, dd].copy()
            q[mask, dd] = alt[mask, dd]
            alt[mask, dd] = qa[mask]
        if flips == 0:
            break
    return q


def _jacobi_ada(Wm, M, fracs):
    """Damped Jacobi-style AdaRound for large row sets: per round, recompute
    G = E @ M once and flip a random `frac` of the improving roundings."""
    q = _q8(Wm)
    alt = _other_candidate(Wm, q)
    E = Wm - q
    Md = np.diag(M).copy()
    rng = np.random.default_rng(0)
    for frac in fracs:
        G = E @ M
        t = q - alt
        np.multiply(G, 2.0 * t, out=G)
        G += (t * t) * Md[None, :]
        mask = G < 0
        if frac < 1.0:
            mask &= rng.random(mask.shape) < frac
        if not mask.any():
            break
        tm = np.where(mask, t, 0.0)
        E += tm
        q -= tm
        alt += tm
    return q


def kernel(features, weight, margins, labels):
    global LAST_RESULT
    from concourse.bass_utils import run_bass_kernel_spmd

    feats = np.asarray(features, np.float32)
    w = np.asarray(weight, np.float32)
    marg = np.asarray(margins, np.float32)
    lab = np.asarray(labels).astype(np.int64)

    nc = _build_nc()

    # --- host prep: normalize, fold S ---
    fn = feats / np.linalg.norm(feats, axis=1, keepdims=True)
    fnT_f16 = np.ascontiguousarray(_to_f16(fn.T).reshape(DBLK, 128, B))

    R = CPC * K  # weight rows per core
    nrm = np.sqrt(np.einsum("ij,ij->i", w, w, dtype=np.float32))
    wS = w * (S / nrm)[:, None]

    # --- AdaRound the fp8 operands (both sides) against each other ---
    Wu = np.concatenate([wS[m * R:m * R + RU] for m in range(NCORES)])
    W8r = _q8(Wu)
    M0 = W8r.T @ W8r
    F8v = _cd_exact(fn, M0)                  # [B, D] f32 values on e4m3 grid
    Mf = F8v.T @ F8v
    W8v = _jacobi_ada(
        Wu, Mf, [0.6, 0.45, 0.35, 0.28, 0.22, 0.17, 0.13, 0.1, 0.08, 0.06]
    )

    # f8[p][P][i][b] = F8v[b, P*256+i*128+p]
    e4 = _e4()
    f8_pack = np.ascontiguousarray(
        F8v.T.astype(e4).reshape(2, 2, 128, B).transpose(2, 0, 1, 3)
    )

    in_maps = []
    for m in range(NCORES):
        # fp8 chunks: [3c+k, d] -> [q, p, k, P, i, j]
        pack8 = np.ascontiguousarray(
            W8v[m * RU:(m + 1) * RU].astype(e4)
            .reshape(NC8, NCHUNK, K, 2, 2, 128)
            .transpose(0, 5, 2, 3, 4, 1)
        ).reshape(NC8, 128, K * 2 * 2 * NCHUNK)
        # fp16 chunks: [3c+k, d] -> [q, p, k, d, j]
        pack16 = np.ascontiguousarray(
            _to_f16(wS[m * R + RU:(m + 1) * R])
            .reshape(NC16, NCHUNK, K, DBLK, 128)
            .transpose(0, 4, 2, 3, 1)
        ).reshape(NC16, 128, K * DBLK * NCHUNK)
        in_maps.append({"w8": pack8, "wt": pack16, "fnT": fnT_f16, "f8": f8_pack})

    _install_profile_hook()
    res = None
    for attempt in range(3):
        try:
            res = run_bass_kernel_spmd(nc, in_maps, list(range(NCORES)))
            break
        except Exception:
            # Rare transient NRT_EXEC_UNIT_UNRECOVERABLE; retry fresh.
            if attempt == 2:
                raise
    LAST_RESULT = res
    outp = np.concatenate(
        [res.results[m]["out"] for m in range(NCORES)], axis=1
    ).astype(np.float32)

    # --- host: exact margin value at each label column ---
    idx3 = (lab[:, None] * K + np.arange(K)[None, :]).reshape(-1)
    W3 = w[idx3]
    W3 = W3 / np.linalg.norm(W3, axis=1, keepdims=True)
    c = np.einsum("bkd,bd->bk", W3.reshape(B, K, D), fn).max(axis=1)
    ms = marg[lab]
    sine = np.sqrt(np.maximum(0.0, 1.0 - c * c))
    phi = np.where(
        c > np.cos(np.pi - ms),
        c * np.cos(ms) - sine * np.sin(ms),
        c - np.sin(np.pi - ms) * ms,
    )
    outp[np.arange(B), lab] = (phi * S).astype(np.float32)
    return outp


# revision 25
# speedup vs baseline: 1.0322x; 1.0322x over previous
"""ArcFace loss with adaptive margins and subcenters, distributed over 8 TRN2 cores.

Problem: features [512, 512] f32, weight [300000, 512] f32 (100000 classes x 3
subcenters), margins [100000] f32, labels [512] int. Output [512, 100000] f32:
S * max_k cos(f, w_{c,k}) everywhere, with the ArcFace margin phi at each
sample's label column.

Strategy (classifier/model parallel, per the class-sharding hint):
  - Host: L2-normalize features and weights, fold the scale S into the weight,
    pack each core's 12500-class shard into a DMA-friendly layout, and compute
    (exactly, in f32) the per-sample label-column value phi.
  - Device (x8, no collectives): stream the packed weight shard from HBM.
    16/25 of each core's class-chunks run as fp8e4m3 matmuls in DoubleRow perf
    mode (256-deep contraction per instruction, measured 1.94-1.97x the fp16
    MAC rate on HW); the other 9/25 run in fp16. Plain RTN fp8 would give
    rel-err 2.9e-2 if applied everywhere; adaptive rounding (coordinate
    descent on the rounding choices against the known counterpart matrix,
    AdaRound-style) plus the fp16 majority lands ~1.8e-2 < 2e-2.
    fp8 and fp16 chunks are interleaved so the constant-rate PSUM-drain
    pipeline (ACT copy + 2 DVE maxes, ~1.35us/block) stays below the mean PE
    cadence. Elementwise max over the 3 subcenters, fp16 cosine shard to HBM
    (f32 stores would saturate the ~358 GB/s HBM port alongside the weight
    stream), host upcasts.
  - Host: concatenate the 8 shards and overwrite the label entries with S*phi.

Per-core PE: 600k column-cycles at fp16 would be 250us; the hybrid needs
9*24k + 16*12k = 408k cycles ~ 170us. HBM traffic ~48MB ~ 135us.
"""

import numpy as np

B = 512            # batch
D = 512            # in_features
C = 100000         # n_classes
K = 3              # subcenters
S = 30.0           # ArcFace scale
NCORES = 8
CPC = C // NCORES  # classes per core = 12500
NCHUNK = 500       # output columns per PSUM tile
CHUNKS = CPC // NCHUNK   # 25
NC8 = 16           # chunks computed in fp8 DoubleRow (cols 0..NC8*500)
NC16 = CHUNKS - NC8
NB = B // 128      # 4 row blocks of the batch
DBLK = D // 128    # 4 contraction blocks (fp16); 2 pairs of 256 (fp8 DR)
RU = NC8 * NCHUNK * K  # ada-rounded weight rows per core

_CACHE = {}
LAST_RESULT = None  # BassKernelResults of the most recent run (for profiling)


def _install_profile_hook():
    """Make `antenv.axon_hooks` importable (concourse imports it when tracing
    is requested via BASS_TRACE) and register the NTFF hook if available."""
    import sys
    import types
    try:
        import antenv
    except ImportError:
        return
    if getattr(antenv, "axon_hooks", None) is not None:
        return
    mod = types.ModuleType("antenv.axon_hooks")
    _hook = [None]
    mod.set_axon_ntff_profile_hook = lambda h: _hook.__setitem__(0, h)
    mod.get_axon_ntff_profile_hook = lambda: _hook[0]
    sys.modules["antenv.axon_hooks"] = mod
    antenv.axon_hooks = mod
    try:
        from trn_agent_boot.trn_boot import _ntff_profile_via_ctypes
        hook = _ntff_profile_via_ctypes("/opt/axon/libaxon_pjrt.so")
        if hook is not None:
            mod.set_axon_ntff_profile_hook(hook)
    except Exception:
        pass


def _build_nc():
    if "nc" in _CACHE:
        return _CACHE["nc"]
    import concourse.bacc as bacc
    import concourse.tile as tile
    from concourse import mybir

    BF = mybir.dt.float16
    E4 = mybir.dt.float8e4
    F32 = mybir.dt.float32
    DRM = mybir.MatmulPerfMode.DoubleRow

    nc = bacc.Bacc("TRN2", target_bir_lowering=False, debug=False, num_devices=NCORES)
    # fp8 weight shard, DoubleRow moving layout:
    #   w8[q][p][((k*2+P)*2+i)*NCHUNK + j] = S * wn[3*(c0+q*500+j)+k, P*256+i*128+p]
    w8 = nc.dram_tensor("w8", [NC8, 128, K * 2 * 2 * NCHUNK], E4, kind="ExternalInput")
    # fp16 packed weight shard (chunks NC8..24):
    #   wt[q][p][(k*DBLK+d)*NCHUNK + j] = S * wn[3*(c0+(NC8+q)*500+j)+k, d*128+p]
    wt = nc.dram_tensor("wt", [NC16, 128, K * DBLK * NCHUNK], BF, kind="ExternalInput")
    # Normalized features, fp16 transposed: fnT[d][p][b] = fn[b, d*128+p]
    fnT = nc.dram_tensor("fnT", [DBLK, 128, B], BF, kind="ExternalInput")
    # fp8 features, DoubleRow stationary layout: f8[p][P][i][b] = fn[b, P*256+i*128+p]
    f8 = nc.dram_tensor("f8", [128, 2, 2, B], E4, kind="ExternalInput")
    # fp16 output halves the store traffic (f32 stores + the weight stream
    # would exceed ~358 GB/s during fp8 chunks); host upcasts to f32.
    out = nc.dram_tensor("out", [B, CPC], BF, kind="ExternalOutput")

    with tile.TileContext(nc, trace_sim=False) as tc:
        with tc.tile_pool(name="fp", bufs=1) as fpool, \
             tc.tile_pool(name="wp", bufs=4) as wpool, \
             tc.tile_pool(name="w8p", bufs=4) as w8pool, \
             tc.tile_pool(name="op", bufs=3) as opool, \
             tc.tile_pool(name="tp", bufs=4) as tpool, \
             tc.tile_pool(name="pp0", bufs=2, space="PSUM") as ppool0, \
             tc.tile_pool(name="pp1", bufs=3, space="PSUM") as ppool1, \
             tc.tile_pool(name="pp2", bufs=3, space="PSUM") as ppool2:
            f_sb = fpool.tile([128, DBLK * B], BF)
            f8_sb = fpool.tile([128, 2, 2, B], E4)  # [p, P, i, b]

            def max_store(ps, b, q, ob_big, last=False):
                # PSUM drain: ACT stages k=0 (the DVE can't read two PSUM
                # banks in one op), DVE does the two maxes (~1.35us/block);
                # the fp8/fp16 interleave keeps the mean PE cadence above it.
                t0 = tpool.tile([128, NCHUNK], F32, tag="t0", name="t0")
                nc.scalar.copy(t0[:], ps[0][:])
                t01 = tpool.tile([128, NCHUNK], BF, tag="t01", name="t01")
                nc.vector.tensor_max(t01[:], t0[:], ps[1][:])
                nc.vector.tensor_max(ob_big[:, b], t01[:], ps[2][:])
                if last:
                    # Final chunk: store per b-block so the kernel tail
                    # overlaps the drain instead of one big end store.
                    nc.sync.dma_start(
                        out[b * 128:(b + 1) * 128, q * NCHUNK:(q + 1) * NCHUNK],
                        ob_big[:, b],
                    )
                elif b == NB - 1:
                    # One batched store per chunk (a dma_start occupies the
                    # issuing engine ~600ns, so fewer+larger stores), on
                    # sync's HWDGE ring; weights ride gpsimd's ring.
                    nc.sync.dma_start(
                        out[:, q * NCHUNK:(q + 1) * NCHUNK]
                        .rearrange("(bb p) c -> p bb c", bb=NB),
                        ob_big[:],
                    )

            def psum_tiles():
                pools = (ppool0, ppool1, ppool2)
                return [
                    pools[k].tile([128, NCHUNK], F32, tag=f"ps{k}", name=f"ps{k}")
                    for k in range(K)
                ]

            def emit_f8(q, last=False):
                w_sb = w8pool.tile([128, K * 2, 2, NCHUNK], E4)
                if q == 0:
                    # First chunk: the first DR matmul needs only the P=0
                    # feature slice + the (k0,P0) weight slice; issue those
                    # first on separate rings, then the rest in consumption
                    # order.
                    w83 = w8[q].rearrange("p (m two j) -> p m two j", m=K * 2, two=2)
                    nc.scalar.dma_start(f8_sb[:, 0], f8[:, 0])
                    nc.sync.dma_start(w_sb[:, 0], w83[:, 0])
                    nc.scalar.dma_start(f8_sb[:, 1], f8[:, 1])
                    nc.gpsimd.dma_start(w_sb[:, 1], w83[:, 1])
                    for m in range(2, K * 2):
                        eng = nc.sync if m % 2 == 0 else nc.gpsimd
                        eng.dma_start(w_sb[:, m], w83[:, m])
                    for d in range(DBLK):
                        nc.scalar.dma_start(f_sb[:, d * B:(d + 1) * B], fnT[d])
                else:
                    nc.gpsimd.dma_start(w_sb[:], w8[q])
                ob_big = opool.tile([128, NB, NCHUNK], BF, tag="ob", name="ob")
                for b in range(NB):
                    ps = psum_tiles()
                    for k in range(K):
                        for P in range(2):
                            lh = f8_sb[:, P, :, b * 128:(b + 1) * 128]
                            rh = w_sb[:, k * 2 + P]
                            nc.tensor.matmul(
                                ps[k][:], lh, rh,
                                start=(P == 0), stop=(P == 1),
                                perf_mode=DRM,
                                skip_group_check=True,
                            )
                    max_store(ps, b, q, ob_big, last)

            def emit_f16(q, last=False):
                w_sb = wpool.tile([128, K * DBLK * NCHUNK], BF)
                nc.gpsimd.dma_start(w_sb[:], wt[q])
                ob_big = opool.tile([128, NB, NCHUNK], BF, tag="ob", name="ob")
                for b in range(NB):
                    ps = psum_tiles()
                    # d-outer / k-inner: the stationary operand (features)
                    # is reused across the 3 subcenter matmuls.
                    for d in range(DBLK):
                        lh = f_sb[:, d * B + b * 128: d * B + (b + 1) * 128]
                        for k in range(K):
                            rh = w_sb[:, (k * DBLK + d) * NCHUNK:(k * DBLK + d + 1) * NCHUNK]
                            nc.tensor.matmul(
                                ps[k][:], lh, rh,
                                start=(d == 0), stop=(d == DBLK - 1),
                                skip_group_check=True,
                            )
                    max_store(ps, b, NC8 + q, ob_big, last)

            # Interleave fp8 (5us PE/chunk) and fp16 (10us) chunks (Bresenham
            # merge) so the constant-rate PSUM drain never falls behind.
            order = []
            acc = 0
            i8 = i16 = 0
            for _ in range(CHUNKS):
                acc += NC16
                if acc >= CHUNKS and i16 < NC16:
                    acc -= CHUNKS
                    order.append(("f16", i16))
                    i16 += 1
                elif i8 < NC8:
                    order.append(("f8", i8))
                    i8 += 1
                else:
                    order.append(("f16", i16))
                    i16 += 1
            for i, (kind, q) in enumerate(order):
                (emit_f8 if kind == "f8" else emit_f16)(q, last=(i == len(order) - 1))
    nc.compile()
    _CACHE["nc"] = nc
    return nc


def _to_f16(x):
    # fp16 storage/compute: same TensorE rate and DMA bytes as bf16, but a
    # 10-bit mantissa -> ~4x less quantization error. All values here are
    # bounded by S=30, far inside fp16 range.
    return np.asarray(x, np.float32).astype(np.float16)


def _e4():
    import ml_dtypes
    return ml_dtypes.float8_e4m3


def _q8(x):
    return x.astype(_e4()).astype(np.float32)


def _other_candidate(x, q):
    """e4m3 grid neighbor of q on the other side of x (toward x)."""
    u = x.astype(_e4()).view(np.uint8).astype(np.int16)
    xf = np.asarray(x, np.float32)
    delta = np.where((xf > q) == (q >= 0), 1, -1).astype(np.int16)
    alt = (u + delta).clip(0, 255).astype(np.uint8).view(_e4()).astype(np.float32)
    return np.where(xf == q, q, alt)


def _cd_exact(F, M, sweeps=5):
    """AdaRound via exact coordinate descent: choose the e4m3 rounding of
    each F entry to minimize tr(E M E^T), E = F - Q, M = gram matrix of the
    counterpart operand. Small row count -> exact sequential CD is cheap."""
    q = _q8(F)
    alt = _other_candidate(F, q)
    E = F - q
    G = E @ M
    Md = np.diag(M).copy()
    for _ in range(sweeps):
        flips = 0
        for dd in range(F.shape[1]):
            t = q[:, dd] - alt[:, dd]
            delta = 2.0 * t * G[:, dd] + t * t * Md[dd]
            mask = delta < 0
            if not mask.any():
                continue
            flips += int(mask.sum())
            tm = np.where(mask, t, 0.0)
            E[:, dd] += tm
            G += np.outer(tm, M[dd])
            qa = q[:, dd].copy()
            q[mask, dd] = alt[mask, dd]
            alt[mask, dd] = qa[mask]
        if flips == 0:
            break
    return q


def _jacobi_ada(Wm, M, fracs):
    """Damped Jacobi-style AdaRound for large row sets: per round, recompute
    G = E @ M once and flip a random `frac` of the improving roundings."""
    q = _q8(Wm)
    alt = _other_candidate(Wm, q)
    E = Wm - q
    Md = np.diag(M).copy()
    rng = np.random.default_rng(0)
    for frac in fracs:
        G = E @ M
        t = q - alt
        np.multiply(G, 2.0 * t, out=G)
        G += (t * t) * Md[None, :]
        mask = G < 0
        if frac < 1.0:
            mask &= rng.random(mask.shape) < frac
        if not mask.any():
            break
        tm = np.where(mask, t, 0.0)
        E += tm
        q -= tm
        alt += tm
    return q


def kernel(features, weight, margins, labels):
    global LAST_RESULT
    from concourse.bass_utils import run_bass_kernel_spmd

    feats = np.asarray(features, np.float32)
    w = np.asarray(weight, np.float32)
    marg = np.asarray(margins, np.float32)
    lab = np.asarray(labels).astype(np.int64)

    nc = _build_nc()

    # --- host prep: normalize, fold S ---
    fn = feats / np.linalg.norm(feats, axis=1, keepdims=True)
    fnT_f16 = np.ascontiguousarray(_to_f16(fn.T).reshape(DBLK, 128, B))

    R = CPC * K  # weight rows per core
    nrm = np.sqrt(np.einsum("ij,ij->i", w, w, dtype=np.float32))
    wS = w * (S / nrm)[:, None]

    # --- AdaRound the fp8 operands (both sides) against each other ---
    Wu = np.concatenate([wS[m * R:m * R + RU] for m in range(NCORES)])
    W8r = _q8(Wu)
    M0 = W8r.T @ W8r
    F8v = _cd_exact(fn, M0)                  # [B, D] f32 values on e4m3 grid
    Mf = F8v.T @ F8v
    W8v = _jacobi_ada(
        Wu, Mf, [0.6, 0.45, 0.35, 0.28, 0.22, 0.17, 0.13, 0.1, 0.08, 0.06]
    )

    # f8[p][P][i][b] = F8v[b, P*256+i*128+p]
    e4 = _e4()
    f8_pack = np.ascontiguousarray(
        F8v.T.astype(e4).reshape(2, 2, 128, B).transpose(2, 0, 1, 3)
    )

    in_maps = []
    for m in range(NCORES):
        # fp8 chunks: [3c+k, d] -> [q, p, k, P, i, j]
        pack8 = np.ascontiguousarray(
            W8v[m * RU:(m + 1) * RU].astype(e4)
            .reshape(NC8, NCHUNK, K, 2, 2, 128)
            .transpose(0, 5, 2, 3, 4, 1)
        ).reshape(NC8, 128, K * 2 * 2 * NCHUNK)
        # fp16 chunks: [3c+k, d] -> [q, p, k, d, j]
        pack16 = np.ascontiguousarray(
            _to_f16(wS[m * R + RU:(m + 1) * R])
            .reshape(NC16, NCHUNK, K, DBLK, 128)
            .transpose(0, 4, 2, 3, 1)
        ).reshape(NC16, 128, K * DBLK * NCHUNK)
        in_maps.append({"w8": pack8, "wt": pack16, "fnT": fnT_f16, "f8": f8_pack})

    _install_profile_hook()
    res = None
    for attempt in range(3):
        try:
            res = run_bass_kernel_spmd(nc, in_maps, list(range(NCORES)))
            break
        except Exception:
            # Rare transient NRT_EXEC_UNIT_UNRECOVERABLE; retry fresh.
            if attempt == 2:
                raise
    LAST_RESULT = res
    outp = np.concatenate(
        [res.results[m]["out"] for m in range(NCORES)], axis=1
    ).astype(np.float32)

    # --- host: exact margin value at each label column ---
    idx3 = (lab[:, None] * K + np.arange(K)[None, :]).reshape(-1)
    W3 = w[idx3]
    W3 = W3 / np.linalg.norm(W3, axis=1, keepdims=True)
    c = np.einsum("bkd,bd->bk", W3.reshape(B, K, D), fn).max(axis=1)
    ms = marg[lab]
    sine = np.sqrt(np.maximum(0.0, 1.0 - c * c))
    phi = np.where(
        c > np.cos(np.pi - ms),
        c * np.cos(ms) - sine * np.sin(ms),
        c - np.sin(np.pi - ms) * ms,
    )
    outp[np.arange(B), lab] = (phi * S).astype(np.float32)
    return outp
